# revision 43
# baseline (speedup 1.0000x reference)
"""Trainium2 Bass kernel for the MANN network (LSTM scan + memory-write scan).

Self-contained: hardcodes all shapes. kernel(**inputs) takes full numpy inputs
and returns the full [128, 40] final memory matrix.

Structure (single-core program, replicated on 8 cores via SPMD):
  Phase A (batch): GXT[p, jj, t] = gate pre-activations from x/y (PE matmuls -> DRAM)
  Per chunk c of 128 steps, section order [B(c)][Cq(c)][D(c-1)][Cu(c)]:
    B(c): 128 sequential LSTM steps (tanh-only ACT, fused sigmoid*mul custom
          DVE ops, fp16 W_hh stationary)
    Cq(c): chunk keys/sigma batch matmuls + per-chunk Gram matrices
          Qn[s,t]=k_s.kn_t and Q2m[s,t]=-k_s.k_t, staged flat on partition 0
    D(c-1): 128 sequential memory steps. M itself is only updated per-chunk;
          within a chunk the matvecs (M@kn, M@k) live in a PSUM accumulator
          U[slot, t] updated by one rank-1 outer product per step
          (ww_t x Gram-row), and row norms are tracked incrementally
          (nh = -0.5*||M_row||^2) with a fused Newton rsqrt refresh.
    Cu(c): U re-init for chunk c from the freshly updated M.
"""

import sys

import numpy as np

# concourse (Bass) lives in the TRN RL repo; make it importable regardless of cwd
for _p in ("/opt/trn_rl_repo", "/root/.axon_site/_ro/trn_rl_repo"):
    try:
        import concourse  # noqa: F401
        break
    except ImportError:
        if _p not in sys.path:
            sys.path.insert(0, _p)

T, D, F, H, NS, KD = 4096, 512, 256, 200, 128, 40
TC = 128                  # steps per chunk
NCH = T // TC             # 32 chunks
G4P = 1024                # padded gate vector (4 gates x 256)
QUAKE_F = 1597463007.0    # 0x5f3759df as float
N_CORES = 8
RN0 = float(1.0 / np.sqrt(KD * 1e-12))   # initial 1/||M_row||, M0 = 1e-6
NH0 = float(-0.5 * KD * 1e-12)           # initial -0.5*||M_row||^2


# ---------------------------------------------------------------- host prep --
def _prep(inputs):
    f32 = np.float32
    x = np.ascontiguousarray(inputs["x_train"], f32)
    y = np.ascontiguousarray(inputs["y_train"], f32)
    W_in = np.asarray(inputs["W_in"], f32)
    b_in = np.asarray(inputs["b_in"], f32)
    W_ih = np.asarray(inputs["W_ih"], f32)
    W_hh = np.asarray(inputs["W_hh"], f32)
    b_ih = np.asarray(inputs["b_ih"], f32)
    b_hh = np.asarray(inputs["b_hh"], f32)
    W_k = np.asarray(inputs["W_k"], f32)
    b_k = np.asarray(inputs["b_k"], f32)
    W_s = np.asarray(inputs["W_s"], f32)
    b_s = np.asarray(inputs["b_s"], f32)

    # Gate reorder (i, f, gg, o) -> (i, f, o, gg); sigmoid gates scaled by 0.5
    # (sigmoid(v) = 0.5*tanh(0.5 v)+0.5), pad each gate 200 -> 256 rows.
    gate_src = [0, 1, 3, 2]
    scale = [0.5, 0.5, 0.5, 1.0]
    b_tot = b_ih + b_hh
    Wtil = np.zeros((G4P, F + 2), f32)   # cols 0:256 = x feats, 256 = y, 257 = bias
    Whhp = np.zeros((G4P, H), f32)
    for g in range(4):
        src = gate_src[g]
        rows = slice(256 * g, 256 * g + H)
        Wtil[rows, 0:F + 1] = scale[g] * W_ih[200 * src:200 * src + H, :]
        Wtil[rows, F + 1] = scale[g] * b_tot[200 * src:200 * src + H]
        Whhp[rows, :] = scale[g] * W_hh[200 * src:200 * src + H, :]

    watil_t = np.ascontiguousarray(Wtil.T).astype(np.float16)  # [258, 1024] fp16
    whhT = np.ascontiguousarray(Whhp.T).astype(np.float16)     # [200, 1024] fp16
    # keys/sigma weights: rows 0:200 hid, 200:224 zero pad, 224 bias; col 40 W_s/2
    wks = np.zeros((225, KD + 1), f32)
    wks[0:H, 0:KD] = W_k
    wks[224, 0:KD] = b_k
    wks[0:H, KD] = 0.5 * W_s[:, 0]
    wks[224, KD] = 0.5 * b_s[0]
    wks = wks.astype(np.float16)

    ysh1 = np.zeros((2, T), np.float16)   # row0 = y_shift, row1 = ones
    ysh1[0, 1:] = y[:-1, 0]
    ysh1[1, :] = 1.0
    return {
        "x16": x.astype(np.float16),
        "ysh1": ysh1,
        "watil_t": watil_t,
        "whht": whhT,
        "wks": wks,
        "w_in16": np.ascontiguousarray(W_in).astype(np.float16),
        "b_in": np.ascontiguousarray(b_in.reshape(2, 128)),   # [m, p] -> load as [128,2]
    }


# ------------------------------------------------------------- bass program --
def build(nc, tc):
    import concourse.bass as bass
    from concourse import mybir
    from concourse.bass import ds
    from concourse.dve_ops import (
        AFFINE_MUL_REDUCE,
        AFFINE_THEN_ADD,
        RECIPROCAL_APPROX_NR,
    )

    f32 = mybir.dt.float32
    f16 = mybir.dt.float16
    u32 = mybir.dt.uint32
    AF = mybir.ActivationFunctionType
    OP = mybir.AluOpType
    X = mybir.AxisListType.X

    x_d = nc.dram_tensor("x16", [T, D], f16, kind="ExternalInput")
    y_d = nc.dram_tensor("ysh1", [2, T], f16, kind="ExternalInput")
    watil_d = nc.dram_tensor("watil_t", [F + 2, G4P], f16, kind="ExternalInput")
    whht_d = nc.dram_tensor("whht", [H, G4P], f16, kind="ExternalInput")
    wks_d = nc.dram_tensor("wks", [225, KD + 1], f16, kind="ExternalInput")
    win_d = nc.dram_tensor("w_in16", [D, F], f16, kind="ExternalInput")
    bin_d = nc.dram_tensor("b_in", [2, 128], f32, kind="ExternalInput")
    m_out = nc.dram_tensor("m_out", [NS, KD], f32, kind="ExternalOutput")
    gxt_d = nc.dram_tensor("gxt", [128, 8, T + TC], f32)  # internal scratch
    qq_dram = nc.dram_tensor("qqd", [2, 128, 256], f16)   # Gram-row regroup bounce

    from contextlib import ExitStack
    stack = ExitStack()

    singles = stack.enter_context(tc.tile_pool(name="singles", bufs=1))

    # ---------------- persistent loop tiles (SBUF) ----------------
    whh_lo = singles.tile([128, G4P], f16)
    whh_hi = singles.tile([72, G4P], f16)
    wks_lo = singles.tile([128, KD + 1], f16)
    wks_hi = singles.tile([97, KD + 1], f16)
    ident128 = singles.tile([128, 128], f32)
    ident1 = singles.tile([1, 1], f32)
    onesm05_40 = singles.tile([40, 128], f32)  # all -0.5: kk broadcast matmul lhsT
    ones1r = singles.tile([1, 128], f32)       # ones row: partition-broadcast lhsT
    fo128 = singles.tile([128, 128], f32)      # all ones: softmax-sum broadcast lhsT

    h16 = singles.tile([128, 2], f16)       # hidden state (col0 = h[0:128], col1 = h[128:200]+pad)
    tg = singles.tile([128, 10], f32)       # cols 0:8 tanh(gates); cols 8:10 = c state
    gsum = singles.tile([128, 8], f32)      # fused sigmoid*operand products
    thc = singles.tile([128, 2], f32)
    amr_scr = singles.tile([128, 1], f32)   # unused accumulator sinks
    amr_scr2 = singles.tile([128, 1], f32)

    Mt = singles.tile([40, 128], f32)       # memory, transposed [key, slot]; per-chunk
    e_all = singles.tile([128, TC + 1], f32)  # col t+1 holds e_t
    ww_all = singles.tile([128, TC + 1], f32)  # col t = write weights of step t
    wwrow16 = singles.tile([1, 128], f16)   # current ww as a partition-0 row
    w2t = singles.tile([128, 1], f32)
    sigrs_bc = singles.tile([128, 1], f32)
    nh = [singles.tile([128, 1], f32, tag=f"nh{p}", name=f"nh{p}") for p in range(2)]
    rn = [singles.tile([128, 1], f32, tag=f"rn{p}", name=f"rn{p}") for p in range(2)]
    rnt = singles.tile([128, 1], f32)
    rn2t = singles.tile([128, 1], f32)
    t2 = singles.tile([128, 1], f32)
    rs_bc = singles.tile([128, 1], f32)     # 1/S broadcast down all partitions
    n2c = singles.tile([128, 1], f32)
    qu1 = singles.tile([128, 1], u32)
    qf1 = singles.tile([128, 1], f32)
    qf2 = singles.tile([128, 1], f32)
    qy0 = singles.tile([128, 1], u32)
    m_sb = singles.tile([NS, KD], f32)
    wwT_sb = singles.tile([128, 128], f32)  # per-chunk write weights, [step, slot]
    ktSB = singles.tile([40, 128], f32)
    ktrm = singles.tile([40, 128], f32)     # -keys^T
    kt2 = singles.tile([40, 128], f32)      # keys^T squared

    # ping-pong chunk tiles
    gx_tile = [singles.tile([128, 8, TC], f32, tag=f"gx{p}", name=f"gx{p}") for p in range(2)]
    hidc_a = [singles.tile([128, TC], f16, tag=f"ha{p}", name=f"ha{p}") for p in range(2)]
    hidc_b = [singles.tile([97, TC], f16, tag=f"hb{p}", name=f"hb{p}") for p in range(2)]
    keysc = [singles.tile([128, KD + 1], f32, tag=f"kc{p}", name=f"kc{p}") for p in range(2)]
    knc = [singles.tile([40, 128], f32, tag=f"kn{p}", name=f"kn{p}") for p in range(2)]
    sigrow = [singles.tile([1, 128], f32, tag=f"sr{p}", name=f"sr{p}") for p in range(2)]
    sig_bcc = [singles.tile([128, TC], f32, tag=f"sb{p}", name=f"sb{p}") for p in range(2)]
    omsig_bcc = [singles.tile([128, TC], f32, tag=f"ob{p}", name=f"ob{p}") for p in range(2)]
    kkm_bcc = [singles.tile([128, TC], f32, tag=f"kk{p}", name=f"kk{p}") for p in range(2)]
    qqh = singles.tile([128, 256], f16)     # Gram-row staging: row s = [Qn[s,:] | Q2m[s,:]]
    # partition-0 ring of Gram rows: dim1 = par*2 + (group%2), 16 rows per group
    qqring = singles.tile([1, 4, 16, 256], f16)
    sigpad = singles.tile([128, 32], f32)
    sigtr = singles.tile([128, 32], f32)

    # ---------------- static init ----------------
    nc.sync.dma_start(whh_lo[:], whht_d[0:128, :])
    nc.sync.dma_start(whh_hi[:], whht_d[128:200, :])
    nc.sync.dma_start(wks_lo[:], wks_d[0:128, :])
    nc.sync.dma_start(wks_hi[:], wks_d[128:225, :])
    nc.vector.memset(ident128[:], 1.0)
    nc.gpsimd.affine_select(ident128[:], ident128[:], [[-1, 128]], OP.is_equal, 0.0,
                            base=0, channel_multiplier=1)
    nc.vector.memset(onesm05_40[:], -0.5)
    nc.vector.memset(ones1r[:], 1.0)
    nc.vector.memset(fo128[:], 1.0)
    nc.vector.memset(h16[:], 0.0)
    nc.vector.memset(tg[:], 0.0)
    nc.vector.memset(Mt[:], 1e-6)
    nc.vector.memset(rs_bc[:], 1.0)
    nc.vector.memset(rn[0][:], RN0)
    nc.vector.memset(rn[1][:], RN0)
    nc.vector.memset(nh[0][:], NH0)
    nc.vector.memset(nh[1][:], NH0)
    nc.vector.memset(e_all[:, 0:1], 0.0)
    nc.vector.memset(e_all[0:1, 0:1], 1.0)
    nc.vector.memset(sigpad[:], 0.0)
    for p in range(2):
        nc.vector.memset(hidc_b[p][:], 0.0)
        nc.vector.memset(hidc_b[p][96:97, :], 1.0)

    # ---------------- phase A: GXT ----------------
    with tc.tile_pool(name="pha1", bufs=1) as pha1, \
         tc.tile_pool(name="pha", bufs=3) as pha, \
         tc.tile_pool(name="pha_ps", bufs=2, space="PSUM") as pha_ps:
        xT = [pha1.tile([128, T], f16, tag=f"xT{k}", name=f"xT{k}") for k in range(4)]
        for k in range(4):
            nc.sync.dma_start(xT[k][:], x_d[:, 128 * k:128 * (k + 1)].rearrange("t d -> d t"))
        win_sb = pha1.tile([128, 4, F], f16)
        nc.sync.dma_start(win_sb[:], win_d.rearrange("(k p) f -> p k f", p=128))
        binc = pha1.tile([128, 2], f32)
        nc.sync.dma_start(binc[:], bin_d.rearrange("m p -> p m"))
        wat0 = pha1.tile([128, G4P], f16)
        wat1 = pha1.tile([128, G4P], f16)
        wat2 = pha1.tile([2, G4P], f16)
        nc.sync.dma_start(wat0[:], watil_d[0:128, :])
        nc.sync.dma_start(wat1[:], watil_d[128:256, :])
        nc.sync.dma_start(wat2[:], watil_d[256:258, :])

        xys0 = pha1.tile([128, T], f16)
        xys1 = pha1.tile([128, T], f16)
        xys2 = pha1.tile([2, T], f16)
        nc.sync.dma_start(xys2[:], y_d[:])

        # xsT = W_in.T @ x.T  (+ b_in)
        for m in range(2):
            dst = xys0 if m == 0 else xys1
            for n in range(8):
                ps = pha_ps.tile([128, 512], f32, tag="psA")
                for k in range(4):
                    nc.tensor.matmul(ps[:], win_sb[:, k, 128 * m:128 * (m + 1)],
                                     xT[k][:, 512 * n:512 * (n + 1)],
                                     start=(k == 0), stop=(k == 3))
                nc.vector.tensor_scalar(dst[:, 512 * n:512 * (n + 1)], ps[:],
                                        binc[:, m:m + 1], None, OP.add)

        # GXT = Wtil_aug.T-slices @ xysT -> DRAM
        for jj in range(8):
            for n in range(8):
                ps = pha_ps.tile([128, 512], f32, tag="psA")
                nc.tensor.matmul(ps[:], wat0[:, 128 * jj:128 * (jj + 1)],
                                 xys0[:, 512 * n:512 * (n + 1)], start=True, stop=False)
                nc.tensor.matmul(ps[:], wat1[:, 128 * jj:128 * (jj + 1)],
                                 xys1[:, 512 * n:512 * (n + 1)], start=False, stop=False)
                nc.tensor.matmul(ps[:], wat2[:, 128 * jj:128 * (jj + 1)],
                                 xys2[:, 512 * n:512 * (n + 1)], start=False, stop=True)
                stg = pha.tile([128, 512], f32, tag="stgA")
                nc.vector.tensor_copy(stg[:], ps[:])
                nc.sync.dma_start(gxt_d[:, jj, 512 * n:512 * (n + 1)], stg[:])

    # loop-phase PSUM (8 banks)
    psingles = stack.enter_context(tc.tile_pool(name="psingles", bufs=1, space="PSUM"))
    U = psingles.tile([128, 256], f32)      # cols 0:128 A=M@kn, 128:256 B=-M@k
    gP = psingles.tile([128, 8], f32)
    kraw = psingles.tile([128, KD + 1], f32)
    seb = psingles.tile([128, 1], f32)      # softmax sum, broadcast to all partitions
    psQ = psingles.tile([128, 128], f32)    # ktr staging / Gram staging / ww^T
    psMD = psingles.tile([40, 128], f32)    # per-chunk M delta
    psMISC = psingles.tile([128, 128], f32) # broadcast staging / final out transpose

    # ---------------- emitters ----------------
    def emit_B_step(c, gx, s):
        par = c % 2
        ha, hb = hidc_a[par], hidc_b[par]
        # gates = gx + Whh @ h; gx enters PSUM via an identity matmul
        nc.tensor.matmul(gP[:], ident128[:], gx[:, :, s], start=True, stop=False)
        for kc in range(2):
            slab = whh_lo if kc == 0 else whh_hi
            rhs = h16[:, 0:1] if kc == 0 else h16[0:72, 1:2]
            for jj in range(8):
                nc.tensor.matmul(gP[:, jj:jj + 1], slab[:, 128 * jj:128 * (jj + 1)],
                                 rhs, start=False, stop=(kc == 1 and jj == 7))
        nc.scalar.activation(tg[:, 0:8], gP[:], AF.Tanh)
        # fused: gsum[0:4] = sigmoid(i,f) * [tanh(gg) | c]
        nc.vector._custom_dve(AFFINE_MUL_REDUCE, out=gsum[:, 0:4], in0=tg[:, 0:4],
                              in1=tg[:, 6:10], s0=0.5, s1=0.5,
                              accum_out=amr_scr[:])
        nc.vector.tensor_add(tg[:, 8:10], gsum[:, 0:2], gsum[:, 2:4])
        nc.scalar.activation(thc[:], tg[:, 8:10], AF.Tanh)
        # fused: h = sigmoid(o) * tanh(c), straight to fp16
        nc.vector._custom_dve(AFFINE_MUL_REDUCE, out=h16[:], in0=tg[:, 4:6],
                              in1=thc[:], s0=0.5, s1=0.5,
                              accum_out=amr_scr2[:])
        nc.gpsimd.tensor_copy(ha[:, s:s + 1], h16[:, 0:1])
        nc.gpsimd.tensor_copy(hb[0:72, s:s + 1], h16[0:72, 1:2])

    def emit_Cq(c):
        """Per-chunk key/sigma/Gram computation (chunk-c data only)."""
        par = c % 2
        nc.tensor.matmul(kraw[:], hidc_a[par][:], wks_lo[:], start=True, stop=False)
        nc.tensor.matmul(kraw[:], hidc_b[par][:], wks_hi[:], start=False, stop=True)
        nc.scalar.activation(keysc[par][:], kraw[:], AF.Tanh)
        nc.vector.tensor_scalar(sigpad[:, 0:1], keysc[par][:, KD:KD + 1], 0.5, 0.5,
                                OP.mult, OP.add)
        nc.vector.transpose(sigtr[:], sigpad[:])
        for i in range(4):
            nc.gpsimd.tensor_copy(sigrow[par][0:1, 32 * i:32 * (i + 1)],
                                  sigtr[32 * i:32 * i + 1, 0:32])
        # sigma / (1-sigma) broadcast down all 128 partitions (column t = step t)
        nc.tensor.matmul(psQ[:], ones1r[:], sigrow[par][:], start=True, stop=True)
        nc.vector.tensor_copy(sig_bcc[par][:], psQ[:])
        nc.vector.tensor_scalar(omsig_bcc[par][:], sig_bcc[par][:], -1.0, 1.0,
                                OP.mult, OP.add)
        # keys^T and derived tiles
        nc.tensor.transpose(psQ[0:KD, :], keysc[par][:, 0:KD], ident128[:])
        nc.vector.tensor_copy(ktSB[:], psQ[0:KD, :])
        nc.scalar.activation(knc[par][:], ktSB[:], AF.Sign)
        nc.vector.tensor_scalar(ktrm[:], ktSB[:], -1.0, None, OP.mult)
        nc.scalar.activation(kt2[:], ktSB[:], AF.Square)
        # -0.5*||k_t||^2 broadcast down all partitions
        nc.tensor.matmul(psQ[:], onesm05_40[:], kt2[:], start=True, stop=True)
        nc.vector.tensor_copy(kkm_bcc[par][:], psQ[:])
        # Gram matrices: Qn[s,t] = k_s.kn_t ; Q2m[s,t] = -k_s.k_t
        nc.tensor.matmul(psQ[:], ktSB[:], knc[par][:], start=True, stop=True)
        nc.vector.tensor_copy(qqh[:, 0:128], psQ[:])
        nc.tensor.matmul(psQ[:], ktSB[:], ktrm[:], start=True, stop=True)
        nc.vector.tensor_copy(qqh[:, 128:256], psQ[:])
        # bounce rows to DRAM; prefetch the first 16-row group to partition 0
        nc.sync.dma_start(qq_dram[par], qqh[:])
        nc.sync.dma_start(qqring[0:1, 2 * par, :, :], qq_dram[par][0:16, :])

    def emit_Cu(c):
        """U re-init for chunk c (after D(c-1) updated Mt and drained U)."""
        par = c % 2
        nc.tensor.matmul(U[:, 0:128], Mt[:], knc[par][:], start=True, stop=True,
                         skip_group_check=True)
        nc.tensor.matmul(U[:, 128:256], Mt[:], ktrm[:], start=True, stop=True,
                         skip_group_check=True)

    ww_psrow = psMISC[0:1, 0:128]

    def emit_ww_prep(c_next, s_next, col_idx):
        """Build write weights for step s_next of chunk c_next from the e just
        produced (e_all[:, col_idx]) and stage the fp16 row on partition 0."""
        parn = c_next % 2
        nc.vector.tensor_scalar(sigrs_bc[:], rs_bc[:], sig_bcc[parn][:, s_next:s_next + 1],
                                None, OP.mult)
        nc.vector.tensor_scalar(ww_all[:, col_idx:col_idx + 1], e_all[:, col_idx:col_idx + 1],
                                sigrs_bc[:], omsig_bcc[parn][:, s_next:s_next + 1],
                                OP.mult, OP.add)
        nc.tensor.transpose(ww_psrow, ww_all[:, col_idx:col_idx + 1], ident128[:])
        nc.vector.tensor_copy(wwrow16[:], ww_psrow)

    def emit_D_step(c, s, cold=False, final=False):
        par = c % 2
        p0, p1 = s % 2, (s + 1) % 2
        g, b = s // 16, s % 16
        if b == 0 and g < 7:
            # prefetch the next 16-row Gram group into the other ring slot
            nc.sync.dma_start(qqring[0:1, 2 * par + (g + 1) % 2, :, :],
                              qq_dram[par][16 * (g + 1):16 * (g + 2), :])
        ww_ap = ww_all[:, s:s + 1]
        # nh' = nh + ww*(-M_t@k_t) - 0.5*ww^2*kk   (U B-half read BEFORE outer)
        nc.vector.tensor_scalar(w2t[:], ww_ap, ww_ap,
                                kkm_bcc[par][:, s:s + 1], OP.mult, OP.mult)
        nc.vector._custom_dve(AFFINE_THEN_ADD, out=nh[p1][:], in0=U[:, 128 + s:129 + s],
                              in1=w2t[:], s0=ww_ap, s1=nh[p0][:])
        if cold:
            # quake rsqrt seed + 3 Newton iterations (chunk 0 only)
            nc.vector.tensor_scalar(n2c[:], nh[p1][:], -2.0, 1e-24, OP.mult, OP.max)
            nc.vector.tensor_scalar(qu1[:], n2c.bitcast(u32)[:], 1, None,
                                    OP.logical_shift_right)
            nc.vector.tensor_copy(qf1[:], qu1[:])
            nc.vector.tensor_scalar(qf2[:], qf1[:], -1.0, QUAKE_F, OP.mult, OP.add)
            nc.vector.tensor_copy(qy0[:], qf2[:])
            nc.vector.tensor_copy(rnt[:], qy0.bitcast(f32)[:])
            nc.vector.tensor_scalar(t2[:], nh[p1][:], rnt[:], -1.0, OP.mult, OP.mult)
            nc.vector._custom_dve(RECIPROCAL_APPROX_NR, out=rn2t[:], in0=t2[:],
                                  in1=rnt[:], s0=1.5)
            nc.vector.tensor_scalar(t2[:], nh[p1][:], rn2t[:], -1.0, OP.mult, OP.mult)
            nc.vector._custom_dve(RECIPROCAL_APPROX_NR, out=rnt[:], in0=t2[:],
                                  in1=rn2t[:], s0=1.5)
            nc.vector.tensor_scalar(t2[:], nh[p1][:], rnt[:], -1.0, OP.mult, OP.mult)
            nc.vector._custom_dve(RECIPROCAL_APPROX_NR, out=rn[p1][:], in0=t2[:],
                                  in1=rnt[:], s0=1.5)
        else:
            # warm: one fused Newton step from previous rn
            nc.vector.tensor_scalar(t2[:], nh[p1][:], rn[p0][:], -1.0,
                                    OP.mult, OP.mult)
            nc.vector._custom_dve(RECIPROCAL_APPROX_NR, out=rn[p1][:], in0=t2[:],
                                  in1=rn[p0][:], s0=1.5)
        # rank-1 Gram-row update of the running matvecs
        slot = 2 * par + g % 2
        nc.tensor.matmul(U[:, 0:128], wwrow16[:], qqring[0:1, slot, b, 0:128],
                         start=False, stop=True, skip_group_check=True)
        nc.tensor.matmul(U[:, 128:256], wwrow16[:], qqring[0:1, slot, b, 128:256],
                         start=False, stop=True, skip_group_check=True)
        # e = exp(u * rn), u read from the A-half of U
        nc.scalar.activation(e_all[:, s + 1:s + 2], U[:, s:s + 1], AF.Exp,
                             scale=rn[p1][:])
        nc.tensor.matmul(seb[:], fo128[:], e_all[:, s + 1:s + 2],
                         start=True, stop=True)
        nc.vector.reciprocal_approx_fast(rs_bc[:], seb[:])
        if not final:
            if s < TC - 1:
                emit_ww_prep(c, s + 1, s + 1)
            else:
                emit_ww_prep(c + 1, 0, TC)

    def emit_D_epilogue(c):
        """Fold the chunk's writes into Mt: M += K^T-weighted write rows."""
        par = c % 2
        nc.tensor.transpose(psQ[:], ww_all[:, 0:TC], ident128[:])
        nc.vector.tensor_copy(wwT_sb[:], psQ[:])
        nc.tensor.matmul(psMD[:], keysc[par][:, 0:KD], wwT_sb[:],
                         start=True, stop=True)
        nc.vector.tensor_add(Mt[:], Mt[:], psMD[:])
        if c < NCH - 1:
            nc.vector.tensor_copy(ww_all[:, 0:1], ww_all[:, TC:TC + 1])

    def emit_section(c_par, iv_expr=None, cD=None, coldD=False, prefetch=True):
        """One chunk section: B(c), Cq(c), D(c-1)+epilogue, Cu(c)."""
        if prefetch:
            if iv_expr is None:
                nc.sync.dma_start(gx_tile[(c_par + 1) % 2][:],
                                  gxt_d[:, :, (c_par + 1) * TC:(c_par + 2) * TC])
            else:
                nc.sync.dma_start(gx_tile[(c_par + 1) % 2][:],
                                  gxt_d[:, :, ds((iv_expr + 1) * TC, TC)])
        for s in range(TC):
            emit_B_step(c_par, gx_tile[c_par % 2], s)
        emit_Cq(c_par)
        if cD is not None:
            for s in range(TC):
                emit_D_step(cD, s, coldD)
            emit_D_epilogue(cD)
        emit_Cu(c_par)

    # ---------------- prologue + loop + epilogue ----------------
    nc.sync.dma_start(gx_tile[0][:], gxt_d[:, :, 0:TC])
    emit_section(0, cD=None)                    # B(0) Cq(0) Cu(0); prefetch chunk 1
    emit_ww_prep(0, 0, 0)                       # seed ww for the very first step
    emit_section(1, cD=0, coldD=True)           # B(1) Cq(1) D(0) cold Cu(1); prefetch 2

    with tc.For_i(0, 14) as i:
        emit_section(2, iv_expr=2 * i + 2, cD=1)
        emit_section(3, iv_expr=2 * i + 3, cD=2)

    emit_section(30, cD=29)                     # prefetches chunk 31
    emit_section(31, cD=30, prefetch=False)
    for s in range(TC):
        emit_D_step(31, s, final=(s == TC - 1))
    emit_D_epilogue(31)

    # output: transpose Mt [40,128] -> [128,40]
    nc.tensor.transpose(psMISC[:, 80:120], Mt[:], ident128[0:40, 0:40])
    nc.vector.tensor_copy(m_sb[:], psMISC[:, 80:120])
    nc.sync.dma_start(m_out[:], m_sb[:])

    stack.close()
    return m_out


_CACHE = {}


def _get_program():
    if "nc" not in _CACHE:
        import concourse.bacc as bacc
        import concourse.tile as tile
        nc = bacc.Bacc("TRN2", target_bir_lowering=False, debug=False)
        with tile.TileContext(nc) as tc:
            build(nc, tc)
        nc.compile()
        _CACHE["nc"] = nc
    return _CACHE["nc"]


def kernel(**inputs) -> np.ndarray:
    from concourse import bass_utils
    nc = _get_program()
    in_map = _prep(inputs)
    res = bass_utils.run_bass_kernel_spmd(
        nc, [dict(in_map) for _ in range(N_CORES)], core_ids=list(range(N_CORES))
    )
    return res.results[0]["m_out"]


# revision 51
# speedup vs baseline: 1.4791x; 1.4791x over previous
"""Trainium2 Bass kernel for the MANN network (LSTM scan + memory-write scan).

Self-contained: hardcodes all shapes. kernel(**inputs) takes full numpy inputs
and returns the full [128, 40] final memory matrix.

Structure (single-core program, replicated on 8 cores via SPMD):
  Phase A (batch): GXT[p, jj, t] = gate pre-activations from x/y (PE matmuls -> DRAM)
  Per chunk c of 128 steps, section order [B(c)][Cq(c)][D(c-1)][Cu(c)]:
    B(c): 128 sequential LSTM steps (tanh-only ACT, fused sigmoid*mul custom
          DVE ops, fp16 W_hh stationary)
    Cq(c): chunk keys/sigma batch matmuls + per-chunk Gram matrices
          Qn[s,t]=k_s.kn_t and Q2m[s,t]=-k_s.k_t, staged flat on partition 0
    D(c-1): 128 sequential memory steps. M itself is only updated per-chunk;
          within a chunk the matvecs (M@kn, M@k) live in a PSUM accumulator
          U[slot, t] updated by one rank-1 outer product per step
          (ww_t x Gram-row), and row norms are tracked incrementally
          (nh = -0.5*||M_row||^2) with a fused Newton rsqrt refresh.
    Cu(c): U re-init for chunk c from the freshly updated M.
"""

import sys

import numpy as np

# concourse (Bass) lives in the TRN RL repo; make it importable regardless of cwd
for _p in ("/opt/trn_rl_repo", "/root/.axon_site/_ro/trn_rl_repo"):
    try:
        import concourse  # noqa: F401
        break
    except ImportError:
        if _p not in sys.path:
            sys.path.insert(0, _p)

T, D, F, H, NS, KD = 4096, 512, 256, 200, 128, 40
TC = 128                  # steps per chunk
NCH = T // TC             # 32 chunks
G4P = 1024                # padded gate vector (4 gates x 256)
QUAKE_F = 1597463007.0    # 0x5f3759df as float
N_CORES = 8
RN0 = float(1.0 / np.sqrt(KD * 1e-12))   # initial 1/||M_row||, M0 = 1e-6
NH0 = float(-0.5 * KD * 1e-12)           # initial -0.5*||M_row||^2


# ---------------------------------------------------------------- host prep --
def _prep(inputs):
    f32 = np.float32
    x = np.ascontiguousarray(inputs["x_train"], f32)
    y = np.ascontiguousarray(inputs["y_train"], f32)
    W_in = np.asarray(inputs["W_in"], f32)
    b_in = np.asarray(inputs["b_in"], f32)
    W_ih = np.asarray(inputs["W_ih"], f32)
    W_hh = np.asarray(inputs["W_hh"], f32)
    b_ih = np.asarray(inputs["b_ih"], f32)
    b_hh = np.asarray(inputs["b_hh"], f32)
    W_k = np.asarray(inputs["W_k"], f32)
    b_k = np.asarray(inputs["b_k"], f32)
    W_s = np.asarray(inputs["W_s"], f32)
    b_s = np.asarray(inputs["b_s"], f32)

    # Gate reorder (i, f, gg, o) -> (i, f, o, gg); sigmoid gates scaled by 0.5
    # (sigmoid(v) = 0.5*tanh(0.5 v)+0.5), pad each gate 200 -> 256 rows.
    gate_src = [0, 1, 3, 2]
    scale = [0.5, 0.5, 0.5, 1.0]
    b_tot = b_ih + b_hh
    Wtil = np.zeros((G4P, F + 2), f32)   # cols 0:256 = x feats, 256 = y, 257 = bias
    Whhp = np.zeros((G4P, H), f32)
    for g in range(4):
        src = gate_src[g]
        rows = slice(256 * g, 256 * g + H)
        Wtil[rows, 0:F + 1] = scale[g] * W_ih[200 * src:200 * src + H, :]
        Wtil[rows, F + 1] = scale[g] * b_tot[200 * src:200 * src + H]
        Whhp[rows, :] = scale[g] * W_hh[200 * src:200 * src + H, :]

    watil_t = np.ascontiguousarray(Wtil.T).astype(np.float16)  # [258, 1024] fp16
    whhT = np.ascontiguousarray(Whhp.T).astype(np.float16)     # [200, 1024] fp16
    # keys/sigma weights: rows 0:200 hid, 200:224 zero pad, 224 bias; col 40 W_s/2
    wks = np.zeros((225, KD + 1), f32)
    wks[0:H, 0:KD] = W_k
    wks[224, 0:KD] = b_k
    wks[0:H, KD] = 0.5 * W_s[:, 0]
    wks[224, KD] = 0.5 * b_s[0]
    wks = wks.astype(np.float16)

    ysh1 = np.zeros((2, T), np.float16)   # row0 = y_shift, row1 = ones
    ysh1[0, 1:] = y[:-1, 0]
    ysh1[1, :] = 1.0
    return {
        "x16": x.astype(np.float16),
        "ysh1": ysh1,
        "watil_t": watil_t,
        "whht": whhT,
        "wks": wks,
        "w_in16": np.ascontiguousarray(W_in).astype(np.float16),
        "b_in": np.ascontiguousarray(b_in.reshape(2, 128)),   # [m, p] -> load as [128,2]
    }


# ------------------------------------------------------------- bass program --
def build(nc, tc):
    import concourse.bass as bass
    from concourse import mybir
    from concourse.bass import ds
    from concourse.dve_ops import (
        AFFINE_MUL_REDUCE,
        AFFINE_THEN_ADD,
        RECIPROCAL_APPROX_NR,
    )

    f32 = mybir.dt.float32
    f16 = mybir.dt.float16
    u32 = mybir.dt.uint32
    AF = mybir.ActivationFunctionType
    OP = mybir.AluOpType
    X = mybir.AxisListType.X

    x_d = nc.dram_tensor("x16", [T, D], f16, kind="ExternalInput")
    y_d = nc.dram_tensor("ysh1", [2, T], f16, kind="ExternalInput")
    watil_d = nc.dram_tensor("watil_t", [F + 2, G4P], f16, kind="ExternalInput")
    whht_d = nc.dram_tensor("whht", [H, G4P], f16, kind="ExternalInput")
    wks_d = nc.dram_tensor("wks", [225, KD + 1], f16, kind="ExternalInput")
    win_d = nc.dram_tensor("w_in16", [D, F], f16, kind="ExternalInput")
    bin_d = nc.dram_tensor("b_in", [2, 128], f32, kind="ExternalInput")
    m_out = nc.dram_tensor("m_out", [NS, KD], f32, kind="ExternalOutput")
    gxt_d = nc.dram_tensor("gxt", [128, 8, T + TC], f32)  # internal scratch
    qq_dram = nc.dram_tensor("qqd", [2, 128, 256], f16)   # Gram-row regroup bounce

    from contextlib import ExitStack
    stack = ExitStack()

    singles = stack.enter_context(tc.tile_pool(name="singles", bufs=1))

    # ---------------- persistent loop tiles (SBUF) ----------------
    whh_lo = singles.tile([128, G4P], f16)
    whh_hi = singles.tile([72, G4P], f16)
    wks_lo = singles.tile([128, KD + 1], f16)
    wks_hi = singles.tile([97, KD + 1], f16)
    ident128 = singles.tile([128, 128], f32)
    ident1 = singles.tile([1, 1], f32)
    onesm05_40 = singles.tile([40, 128], f32)  # all -0.5: kk broadcast matmul lhsT
    ones1r = singles.tile([1, 128], f32)       # ones row: partition-broadcast lhsT
    fo128 = singles.tile([128, 128], f32)      # all ones: softmax-sum broadcast lhsT

    h16 = singles.tile([128, 2], f16)       # hidden state (col0 = h[0:128], col1 = h[128:200]+pad)
    tg = singles.tile([128, 10], f32)       # cols 0:8 tanh(gates); cols 8:10 = c state
    gsum = singles.tile([128, 8], f32)      # fused sigmoid*operand products
    thc = singles.tile([128, 2], f32)
    amr_scr = singles.tile([128, 1], f32)   # unused accumulator sinks
    amr_scr2 = singles.tile([128, 1], f32)

    Mt = singles.tile([40, 128], f32)       # memory, transposed [key, slot]; per-chunk
    e_all = singles.tile([128, TC + 1], f32)  # col t+1 holds e_t
    ww_all = singles.tile([128, TC + 1], f32)  # col t = write weights of step t
    wwrow16 = singles.tile([1, 128], f16)   # current ww as a partition-0 row
    w2t = singles.tile([128, 1], f32)
    sigrs_bc = singles.tile([128, 1], f32)
    rs_bc = singles.tile([128, 1], f32)     # 1/S broadcast down all partitions
    nh = [singles.tile([128, 1], f32, tag=f"nh{p}", name=f"nh{p}") for p in range(2)]
    rn = [singles.tile([128, 1], f32, tag=f"rn{p}", name=f"rn{p}") for p in range(2)]
    rnt = singles.tile([128, 1], f32)
    rn2t = singles.tile([128, 1], f32)
    t2 = singles.tile([128, 1], f32)
    n2c = singles.tile([128, 1], f32)
    qu1 = singles.tile([128, 1], u32)
    qf1 = singles.tile([128, 1], f32)
    qf2 = singles.tile([128, 1], f32)
    qy0 = singles.tile([128, 1], u32)
    m_sb = singles.tile([NS, KD], f32)
    wwT_sb = singles.tile([128, 128], f32)  # per-chunk write weights, [step, slot]
    ktSB = singles.tile([40, 128], f32)
    ktrm = singles.tile([40, 128], f32)     # -keys^T
    kt2 = singles.tile([40, 128], f32)      # keys^T squared

    # ping-pong chunk tiles
    gx_tile = [singles.tile([128, 8, TC], f32, tag=f"gx{p}", name=f"gx{p}") for p in range(2)]
    hidc_a = [singles.tile([128, TC], f16, tag=f"ha{p}", name=f"ha{p}") for p in range(2)]
    hidc_b = [singles.tile([97, TC], f16, tag=f"hb{p}", name=f"hb{p}") for p in range(2)]
    keysc = [singles.tile([128, KD + 1], f32, tag=f"kc{p}", name=f"kc{p}") for p in range(2)]
    knc = [singles.tile([40, 128], f32, tag=f"kn{p}", name=f"kn{p}") for p in range(2)]
    sigrow = [singles.tile([1, 128], f32, tag=f"sr{p}", name=f"sr{p}") for p in range(2)]
    sig_bcc = [singles.tile([128, TC], f32, tag=f"sb{p}", name=f"sb{p}") for p in range(2)]
    omsig_bcc = [singles.tile([128, TC], f32, tag=f"ob{p}", name=f"ob{p}") for p in range(2)]
    kkm_bcc = [singles.tile([128, TC], f32, tag=f"kk{p}", name=f"kk{p}") for p in range(2)]
    qqh = singles.tile([128, 256], f16)     # Gram-row staging: row s = [Qn[s,:] | Q2m[s,:]]
    # partition-0 ring of Gram rows: dim1 = par*2 + (group%2), 16 rows per group
    qqring = singles.tile([1, 4, 16, 256], f16)
    sigpad = singles.tile([128, 32], f32)
    sigtr = singles.tile([128, 32], f32)

    # ---------------- static init ----------------
    nc.sync.dma_start(whh_lo[:], whht_d[0:128, :])
    nc.sync.dma_start(whh_hi[:], whht_d[128:200, :])
    nc.sync.dma_start(wks_lo[:], wks_d[0:128, :])
    nc.sync.dma_start(wks_hi[:], wks_d[128:225, :])
    nc.vector.memset(ident128[:], 1.0)
    nc.gpsimd.affine_select(ident128[:], ident128[:], [[-1, 128]], OP.is_equal, 0.0,
                            base=0, channel_multiplier=1)
    nc.vector.memset(onesm05_40[:], -0.5)
    nc.vector.memset(ones1r[:], 1.0)
    nc.vector.memset(fo128[:], 1.0)
    nc.vector.memset(rs_bc[:], 1.0)
    nc.vector.memset(h16[:], 0.0)
    nc.vector.memset(tg[:], 0.0)
    nc.vector.memset(Mt[:], 1e-6)
    nc.vector.memset(rn[0][:], RN0)
    nc.vector.memset(rn[1][:], RN0)
    nc.vector.memset(nh[0][:], NH0)
    nc.vector.memset(nh[1][:], NH0)
    nc.vector.memset(e_all[:, 0:1], 0.0)
    nc.vector.memset(e_all[0:1, 0:1], 1.0)
    nc.vector.memset(sigpad[:], 0.0)
    for p in range(2):
        nc.vector.memset(hidc_b[p][:], 0.0)
        nc.vector.memset(hidc_b[p][96:97, :], 1.0)

    # ---------------- phase A tiles (interleaved with the first sections) ----
    xT = [singles.tile([128, T], f16, tag=f"xT{k}", name=f"xT{k}") for k in range(4)]
    for k in range(4):
        nc.sync.dma_start(xT[k][:], x_d[:, 128 * k:128 * (k + 1)].rearrange("t d -> d t"))
    win_sb = singles.tile([128, 4, F], f16)
    nc.sync.dma_start(win_sb[:], win_d.rearrange("(k p) f -> p k f", p=128))
    binc = singles.tile([128, 2], f32)
    nc.sync.dma_start(binc[:], bin_d.rearrange("m p -> p m"))
    wat0 = singles.tile([128, G4P], f16)
    wat1 = singles.tile([128, G4P], f16)
    wat2 = singles.tile([2, G4P], f16)
    nc.sync.dma_start(wat0[:], watil_d[0:128, :])
    nc.sync.dma_start(wat1[:], watil_d[128:256, :])
    nc.sync.dma_start(wat2[:], watil_d[256:258, :])
    xys0 = singles.tile([128, T], f16)
    xys1 = singles.tile([128, T], f16)
    xys2 = singles.tile([2, T], f16)
    nc.sync.dma_start(xys2[:], y_d[:])
    stgA = [singles.tile([128, 512], f32, tag=f"stg{k}", name=f"stg{k}")
            for k in range(2)]

    # loop-phase PSUM (8 banks)
    psingles = stack.enter_context(tc.tile_pool(name="psingles", bufs=1, space="PSUM"))
    U = psingles.tile([128, 256], f32)      # cols 0:128 A=M@kn, 128:256 B=-M@k
    gP = psingles.tile([128, 8], f32)
    kraw = psingles.tile([128, KD + 1], f32)
    seb = psingles.tile([128, 1], f32)      # softmax sum, broadcast to all partitions
    psQ = psingles.tile([128, 128], f32)    # ktr staging / Gram staging / ww^T
    psMD = psingles.tile([40, 128], f32)    # per-chunk M delta
    psMISC = psingles.tile([128, 128], f32) # broadcast staging / final out transpose
    psA = psingles.tile([128, 512], f32)    # phase-A accumulator

    def emit_A_block(n):
        """One 512-step slice of the gate-preactivation batch (phase A)."""
        # xsT = W_in.T @ x.T (+ b_in) for this slice
        for m in range(2):
            dst = xys0 if m == 0 else xys1
            for k in range(4):
                nc.tensor.matmul(psA[:], win_sb[:, k, 128 * m:128 * (m + 1)],
                                 xT[k][:, 512 * n:512 * (n + 1)],
                                 start=(k == 0), stop=(k == 3))
            nc.vector.tensor_scalar(dst[:, 512 * n:512 * (n + 1)], psA[:],
                                    binc[:, m:m + 1], None, OP.add)
        # GXT = Wtil_aug.T-slices @ xysT -> DRAM
        for jj in range(8):
            nc.tensor.matmul(psA[:], wat0[:, 128 * jj:128 * (jj + 1)],
                             xys0[:, 512 * n:512 * (n + 1)], start=True, stop=False)
            nc.tensor.matmul(psA[:], wat1[:, 128 * jj:128 * (jj + 1)],
                             xys1[:, 512 * n:512 * (n + 1)], start=False, stop=False)
            nc.tensor.matmul(psA[:], wat2[:, 128 * jj:128 * (jj + 1)],
                             xys2[:, 512 * n:512 * (n + 1)], start=False, stop=True)
            stg = stgA[jj % 2]
            nc.vector.tensor_copy(stg[:], psA[:])
            nc.sync.dma_start(gxt_d[:, jj, 512 * n:512 * (n + 1)], stg[:])

    # ---------------- emitters ----------------
    def emit_B_step(c, gx, s):
        par = c % 2
        ha, hb = hidc_a[par], hidc_b[par]
        # gates = gx + Whh @ h; gx enters PSUM via an identity matmul
        nc.tensor.matmul(gP[:], ident128[:], gx[:, :, s], start=True, stop=False)
        for kc in range(2):
            slab = whh_lo if kc == 0 else whh_hi
            rhs = h16[:, 0:1] if kc == 0 else h16[0:72, 1:2]
            for jj in range(8):
                nc.tensor.matmul(gP[:, jj:jj + 1], slab[:, 128 * jj:128 * (jj + 1)],
                                 rhs, start=False, stop=(kc == 1 and jj == 7))
        nc.scalar.activation(tg[:, 0:8], gP[:], AF.Tanh)
        # fused: gsum[0:4] = sigmoid(i,f) * [tanh(gg) | c]
        nc.vector._custom_dve(AFFINE_MUL_REDUCE, out=gsum[:, 0:4], in0=tg[:, 0:4],
                              in1=tg[:, 6:10], s0=0.5, s1=0.5,
                              accum_out=amr_scr[:])
        nc.vector.tensor_add(tg[:, 8:10], gsum[:, 0:2], gsum[:, 2:4])
        nc.scalar.activation(thc[:], tg[:, 8:10], AF.Tanh)
        # fused: h = sigmoid(o) * tanh(c), straight to fp16
        nc.vector._custom_dve(AFFINE_MUL_REDUCE, out=h16[:], in0=tg[:, 4:6],
                              in1=thc[:], s0=0.5, s1=0.5,
                              accum_out=amr_scr2[:])
        nc.gpsimd.tensor_copy(ha[:, s:s + 1], h16[:, 0:1])
        nc.gpsimd.tensor_copy(hb[0:72, s:s + 1], h16[0:72, 1:2])

    def emit_Cq(c):
        """Per-chunk key/sigma/Gram computation (chunk-c data only)."""
        par = c % 2
        nc.tensor.matmul(kraw[:], hidc_a[par][:], wks_lo[:], start=True, stop=False)
        nc.tensor.matmul(kraw[:], hidc_b[par][:], wks_hi[:], start=False, stop=True)
        nc.scalar.activation(keysc[par][:], kraw[:], AF.Tanh)
        nc.vector.tensor_scalar(sigpad[:, 0:1], keysc[par][:, KD:KD + 1], 0.5, 0.5,
                                OP.mult, OP.add)
        nc.vector.transpose(sigtr[:], sigpad[:])
        for i in range(4):
            nc.gpsimd.tensor_copy(sigrow[par][0:1, 32 * i:32 * (i + 1)],
                                  sigtr[32 * i:32 * i + 1, 0:32])
        # sigma / (1-sigma) broadcast down all 128 partitions (column t = step t)
        nc.tensor.matmul(psQ[:], ones1r[:], sigrow[par][:], start=True, stop=True)
        nc.vector.tensor_copy(sig_bcc[par][:], psQ[:])
        nc.vector.tensor_scalar(omsig_bcc[par][:], sig_bcc[par][:], -1.0, 1.0,
                                OP.mult, OP.add)
        # keys^T and derived tiles
        nc.tensor.transpose(psQ[0:KD, :], keysc[par][:, 0:KD], ident128[:])
        nc.vector.tensor_copy(ktSB[:], psQ[0:KD, :])
        nc.scalar.activation(knc[par][:], ktSB[:], AF.Sign)
        nc.vector.tensor_scalar(ktrm[:], ktSB[:], -1.0, None, OP.mult)
        nc.scalar.activation(kt2[:], ktSB[:], AF.Square)
        # -0.5*||k_t||^2 broadcast down all partitions
        nc.tensor.matmul(psQ[:], onesm05_40[:], kt2[:], start=True, stop=True)
        nc.vector.tensor_copy(kkm_bcc[par][:], psQ[:])
        # Gram matrices: Qn[s,t] = k_s.kn_t ; Q2m[s,t] = -k_s.k_t
        nc.tensor.matmul(psQ[:], ktSB[:], knc[par][:], start=True, stop=True)
        nc.vector.tensor_copy(qqh[:, 0:128], psQ[:])
        nc.tensor.matmul(psQ[:], ktSB[:], ktrm[:], start=True, stop=True)
        nc.vector.tensor_copy(qqh[:, 128:256], psQ[:])
        # bounce rows to DRAM; prefetch the first 16-row group to partition 0
        nc.sync.dma_start(qq_dram[par], qqh[:])
        nc.sync.dma_start(qqring[0:1, 2 * par, :, :], qq_dram[par][0:16, :])

    def emit_Cu(c):
        """U re-init for chunk c (after D(c-1) updated Mt and drained U)."""
        par = c % 2
        nc.tensor.matmul(U[:, 0:128], Mt[:], knc[par][:], start=True, stop=True,
                         skip_group_check=True)
        nc.tensor.matmul(U[:, 128:256], Mt[:], ktrm[:], start=True, stop=True,
                         skip_group_check=True)

    ww_psrow = psMISC[0:1, 0:128]

    def emit_ww_prep(c_next, s_next, col_idx):
        """Build write weights for step s_next of chunk c_next from the e just
        produced (e_all[:, col_idx]) and stage the fp16 row on partition 0.
        The transpose runs on e itself (no rs dependency); the sigma*rs affine
        is applied on the row and the column independently."""
        parn = c_next % 2
        nc.tensor.transpose(ww_psrow, e_all[:, col_idx:col_idx + 1], ident128[:])
        nc.vector.tensor_scalar(sigrs_bc[:], rs_bc[:], sig_bcc[parn][:, s_next:s_next + 1],
                                None, OP.mult)
        nc.vector.tensor_scalar(ww_all[:, col_idx:col_idx + 1], e_all[:, col_idx:col_idx + 1],
                                sigrs_bc[:], omsig_bcc[parn][:, s_next:s_next + 1],
                                OP.mult, OP.add)
        nc.vector.tensor_scalar(wwrow16[:], ww_psrow, sigrs_bc[0:1, 0:1],
                                omsig_bcc[parn][0:1, s_next:s_next + 1],
                                OP.mult, OP.add)

    def emit_D_step(c, s, cold=False, final=False):
        par = c % 2
        p0, p1 = s % 2, (s + 1) % 2
        g, b = s // 16, s % 16
        if b == 0 and g < 7:
            # prefetch the next 16-row Gram group into the other ring slot
            nc.sync.dma_start(qqring[0:1, 2 * par + (g + 1) % 2, :, :],
                              qq_dram[par][16 * (g + 1):16 * (g + 2), :])
        ww_ap = ww_all[:, s:s + 1]
        # nh' = nh + ww*(-M_t@k_t) - 0.5*ww^2*kk   (U B-half read BEFORE outer)
        nc.vector.tensor_scalar(w2t[:], ww_ap, ww_ap,
                                kkm_bcc[par][:, s:s + 1], OP.mult, OP.mult)
        nc.vector._custom_dve(AFFINE_THEN_ADD, out=nh[p1][:], in0=U[:, 128 + s:129 + s],
                              in1=w2t[:], s0=ww_ap, s1=nh[p0][:])
        if cold:
            # quake rsqrt seed + 3 Newton iterations (chunk 0 only)
            nc.vector.tensor_scalar(n2c[:], nh[p1][:], -2.0, 1e-24, OP.mult, OP.max)
            nc.vector.tensor_scalar(qu1[:], n2c.bitcast(u32)[:], 1, None,
                                    OP.logical_shift_right)
            nc.vector.tensor_copy(qf1[:], qu1[:])
            nc.vector.tensor_scalar(qf2[:], qf1[:], -1.0, QUAKE_F, OP.mult, OP.add)
            nc.vector.tensor_copy(qy0[:], qf2[:])
            nc.vector.tensor_copy(rnt[:], qy0.bitcast(f32)[:])
            nc.vector.tensor_scalar(t2[:], nh[p1][:], rnt[:], -1.0, OP.mult, OP.mult)
            nc.vector._custom_dve(RECIPROCAL_APPROX_NR, out=rn2t[:], in0=t2[:],
                                  in1=rnt[:], s0=1.5)
            nc.vector.tensor_scalar(t2[:], nh[p1][:], rn2t[:], -1.0, OP.mult, OP.mult)
            nc.vector._custom_dve(RECIPROCAL_APPROX_NR, out=rnt[:], in0=t2[:],
                                  in1=rn2t[:], s0=1.5)
            nc.vector.tensor_scalar(t2[:], nh[p1][:], rnt[:], -1.0, OP.mult, OP.mult)
            nc.vector._custom_dve(RECIPROCAL_APPROX_NR, out=rn[p1][:], in0=t2[:],
                                  in1=rnt[:], s0=1.5)
        else:
            # warm: one fused Newton step from previous rn
            nc.vector.tensor_scalar(t2[:], nh[p1][:], rn[p0][:], -1.0,
                                    OP.mult, OP.mult)
            nc.vector._custom_dve(RECIPROCAL_APPROX_NR, out=rn[p1][:], in0=t2[:],
                                  in1=rn[p0][:], s0=1.5)
        # rank-1 Gram-row update of the running matvecs
        slot = 2 * par + g % 2
        nc.tensor.matmul(U[:, 0:128], wwrow16[:], qqring[0:1, slot, b, 0:128],
                         start=False, stop=True, skip_group_check=True)
        nc.tensor.matmul(U[:, 128:256], wwrow16[:], qqring[0:1, slot, b, 128:256],
                         start=False, stop=True, skip_group_check=True)
        # e = exp(u * rn), u read from the A-half of U
        nc.scalar.activation(e_all[:, s + 1:s + 2], U[:, s:s + 1], AF.Exp,
                             scale=rn[p1][:])
        nc.tensor.matmul(seb[:], fo128[:], e_all[:, s + 1:s + 2],
                         start=True, stop=True)
        nc.vector.reciprocal_approx_fast(rs_bc[:], seb[:])
        if not final:
            if s < TC - 1:
                emit_ww_prep(c, s + 1, s + 1)
            else:
                emit_ww_prep(c + 1, 0, TC)

    def emit_D_epilogue(c):
        """Fold the chunk's writes into Mt: M += K^T-weighted write rows."""
        par = c % 2
        nc.tensor.transpose(psQ[:], ww_all[:, 0:TC], ident128[:])
        nc.vector.tensor_copy(wwT_sb[:], psQ[:])
        nc.tensor.matmul(psMD[:], keysc[par][:, 0:KD], wwT_sb[:],
                         start=True, stop=True)
        nc.vector.tensor_add(Mt[:], Mt[:], psMD[:])
        if c < NCH - 1:
            nc.vector.tensor_copy(ww_all[:, 0:1], ww_all[:, TC:TC + 1])

    def emit_section(c_par, iv_expr=None, cD=None, coldD=False, prefetch=True):
        """One chunk section: B(c), Cq(c), D(c-1)+epilogue, Cu(c)."""
        if prefetch:
            if iv_expr is None:
                nc.sync.dma_start(gx_tile[(c_par + 1) % 2][:],
                                  gxt_d[:, :, (c_par + 1) * TC:(c_par + 2) * TC])
            else:
                nc.sync.dma_start(gx_tile[(c_par + 1) % 2][:],
                                  gxt_d[:, :, ds((iv_expr + 1) * TC, TC)])
        for s in range(TC):
            emit_B_step(c_par, gx_tile[c_par % 2], s)
        emit_Cq(c_par)
        if cD is not None:
            for s in range(TC):
                emit_D_step(cD, s, coldD)
            emit_D_epilogue(cD)
        emit_Cu(c_par)

    # ---------------- prologue + loop + epilogue ----------------
    emit_A_block(0)
    nc.sync.dma_start(gx_tile[0][:], gxt_d[:, :, 0:TC])
    emit_A_block(1)
    emit_section(0, cD=None)                    # B(0) Cq(0) Cu(0); prefetch chunk 1
    emit_ww_prep(0, 0, 0)                       # seed ww for the very first step
    emit_A_block(2)
    emit_A_block(3)
    emit_A_block(4)
    emit_section(1, cD=0, coldD=True)           # B(1) Cq(1) D(0) cold Cu(1); prefetch 2
    emit_A_block(5)
    emit_A_block(6)
    emit_A_block(7)

    with tc.For_i(0, 14) as i:
        emit_section(2, iv_expr=2 * i + 2, cD=1)
        emit_section(3, iv_expr=2 * i + 3, cD=2)

    emit_section(30, cD=29)                     # prefetches chunk 31
    emit_section(31, cD=30, prefetch=False)
    for s in range(TC):
        emit_D_step(31, s, final=(s == TC - 1))
    emit_D_epilogue(31)

    # output: transpose Mt [40,128] -> [128,40]
    nc.tensor.transpose(psMISC[:, 80:120], Mt[:], ident128[0:40, 0:40])
    nc.vector.tensor_copy(m_sb[:], psMISC[:, 80:120])
    nc.sync.dma_start(m_out[:], m_sb[:])

    stack.close()
    return m_out


_CACHE = {}


def _get_program():
    if "nc" not in _CACHE:
        import concourse.bacc as bacc
        import concourse.tile as tile
        nc = bacc.Bacc("TRN2", target_bir_lowering=False, debug=False)
        with tile.TileContext(nc) as tc:
            build(nc, tc)
        nc.compile()
        _CACHE["nc"] = nc
    return _CACHE["nc"]


def kernel(**inputs) -> np.ndarray:
    from concourse import bass_utils
    nc = _get_program()
    in_map = _prep(inputs)
    res = bass_utils.run_bass_kernel_spmd(
        nc, [dict(in_map) for _ in range(N_CORES)], core_ids=list(range(N_CORES))
    )
    return res.results[0]["m_out"]
